# revision 49
# baseline (speedup 1.0000x reference)
"""Distributed 3-layer GAT kernel for Trainium2 (8 NeuronCores), v2.

Sharding: nodes partitioned contiguously (N/8 per core); each edge is owned by
the core owning its dst node.  Per layer: each core computes the feature-table
rows for its own nodes (h = x @ W.T, the attention scalar s = h.a_src, a
constant-one z column), writes them as fp8 256B rows, and AllGathers the
table; edges are then processed in supergroups: dma_gather of source rows
(256B fp8), one-hot tile masks p16 = (dstrel == iota) in [edge, tile, slot]
layout, d-lookup via pd = p16*d and a contiguous reduce, attention weights
ee = exp(leakyrelu(s_src + d_dst)) (leaky on DVE, exp on ACT - the only ACT
op, keeping its function table warm), and scatter-add via ee-scaled one-hot
matmuls (fp16 x fp8) into per-frame (128 dst nodes) PSUM accumulators;
out = num / z.  Global add-pool via one-hot matmul + AllReduce.

All per-edge index structure (gather indices, dstrel, static tile schedule)
is host-precomputed from edge_index; the same instruction stream runs SPMD on
all 8 cores, per-core variation enters only through data.
"""
import math
from dataclasses import dataclass, field

import numpy as np

import concourse.bacc as bacc
import concourse.tile as tile
from concourse import bass, mybir
from concourse.bass_utils import run_bass_kernel_spmd
from concourse.masks import make_identity

FP32 = mybir.dt.float32
FP16 = mybir.dt.float16
FP8 = mybir.dt.float8e4
I16 = mybir.dt.int16

NEG_SLOPE_ATT = 0.2
NEG_SLOPE_ACT = 0.01


@dataclass
class GATConfig:
    n_nodes: int = 50000
    n_edges: int = 800000
    dim: int = 128
    n_graphs: int = 512
    n_cores: int = 8
    frame: int = 64             # dst window per scatter accumulation
    sg_tile_budget: int = 32    # max tiles per supergroup (gather batch)
    h_major: bool = True        # order sg tiles half-major (2 gathers per sg)
    single_packet: bool = False
    trim_pads: bool = False     # per-core num_idxs_reg from SBUF counts
    g16_bufs: int = 3
    big_runs: bool = False      # merge same-half gather runs across frames

    @property
    def nloc(self):
        return self.n_nodes // self.n_cores

    @property
    def n_frames(self):
        return math.ceil(self.nloc / self.frame)

    @property
    def node_tiles(self):
        return math.ceil(self.nloc / 128)

    @property
    def nloc_pad(self):
        return self.node_tiles * 128

    @property
    def split_i(self):
        # tile-aligned split of each core's shard into A/B halves; keeps both
        # gather tables under the int16 index limit of dma_gather
        return (self.nloc // 2 // 128) * 128

    @property
    def rows_a(self):
        return self.n_cores * self.split_i

    @property
    def rows_b(self):
        return self.n_cores * (self.nloc - self.split_i)

    @property
    def rowlen(self):
        # fp8 elems per table row: h(128) | s | one | pad; 256B rows satisfy
        # the dma_gather 256B-multiple constraint with no re-pad pass
        return 256


@dataclass
class EdgePlan:
    tiles: list = field(default_factory=list)            # per tile: (frame, half)
    frame_spans: dict = field(default_factory=dict)      # f -> [(t0, n), ...]
    sgs: list = field(default_factory=list)              # (t_lo, t_hi, runs, f_lo, f_hi)
    run_group: list = field(default_factory=list)        # per run: (f, h)
    idx16: list = field(default_factory=list)            # per core [128, T*8] int16
    dstrel: list = field(default_factory=list)           # per core [128, T] fp16
    gcnt: list = field(default_factory=list)             # per core [1, nruns] int32


def build_edge_plan(cfg: GATConfig, edge_index: np.ndarray) -> EdgePlan:
    N, C, NL, F = cfg.n_nodes, cfg.n_cores, cfg.nloc, cfg.frame
    src = edge_index[0].astype(np.int64)
    dst = edge_index[1].astype(np.int64)
    loops = np.arange(N, dtype=np.int64)
    src = np.concatenate([src, loops])
    dst = np.concatenate([dst, loops])

    n_frames = cfg.n_frames
    SI = cfg.split_i
    src_core = src // NL
    src_i = src - src_core * NL
    src_hi = (src_i >= SI).astype(np.int64)
    gidx = np.where(src_hi == 0, src_core * SI + src_i,
                    src_core * (NL - SI) + (src_i - SI))
    assert gidx.max() < 32768
    groups = [[[None, None] for _ in range(n_frames)] for _ in range(C)]
    core_of = dst // NL
    dst_loc = dst - core_of * NL
    frame_of = dst_loc // F
    for c in range(C):
        m = core_of == c
        s_c, dl_c, f_c = gidx[m], dst_loc[m], frame_of[m]
        hi_c = src_hi[m]
        order = np.lexsort((dl_c, hi_c, f_c))
        s_c, dl_c, f_c, hi_c = s_c[order], dl_c[order], f_c[order], hi_c[order]
        key = f_c * 2 + hi_c
        bounds = np.searchsorted(key, np.arange(2 * n_frames + 1))
        for f in range(n_frames):
            for h in range(2):
                lo, hi = bounds[2 * f + h], bounds[2 * f + h + 1]
                groups[c][f][h] = (s_c[lo:hi], dl_c[lo:hi])

    plan = EdgePlan()
    # per-(frame, half) tile counts (max across cores)
    ntiles = [[0, 0] for _ in range(n_frames)]
    for f in range(n_frames):
        for h in range(2):
            cnt = max(len(groups[c][f][h][0]) for c in range(C))
            ntiles[f][h] = math.ceil(cnt / 128)

    # supergroups span whole frames; tiles within an sg are ordered h-major
    # (all half-0 tiles first, then half-1) so each sg needs only TWO
    # dma_gather calls - the SWDGE per-call overhead dominates otherwise.
    f_lo = 0
    while f_lo < n_frames:
        f_hi = f_lo
        tot = 0
        while f_hi < n_frames and tot + sum(ntiles[f_hi]) <= cfg.sg_tile_budget:
            tot += sum(ntiles[f_hi])
            f_hi += 1
        assert f_hi > f_lo, "single frame exceeds sg_tile_budget"
        t_lo = len(plan.tiles)
        order = ([(h, f) for h in range(2) for f in range(f_lo, f_hi)]
                 if cfg.h_major else
                 [(h, f) for f in range(f_lo, f_hi) for h in range(2)])
        # one gather run per (frame, half) group: each core's pad slots form a
        # contiguous tail of the group, so -1 indices there are skipped by
        # dma_gather ("negative indices at the end are ignored")
        runs = []
        for (h, f) in order:
            t0 = len(plan.tiles)
            for _ in range(ntiles[f][h]):
                plan.tiles.append((f, h))
            if ntiles[f][h]:
                spans = plan.frame_spans.setdefault(f, [])
                if spans and spans[-1][0] + spans[-1][1] == t0:
                    spans[-1] = (spans[-1][0], spans[-1][1] + ntiles[f][h])
                else:
                    spans.append((t0, ntiles[f][h]))
                runs.append((h, t0, len(plan.tiles)))
                plan.run_group.append((f, h))
        if cfg.big_runs:
            merged = []
            for (h, r_lo, r_hi) in runs:
                if merged and merged[-1][0] == h and merged[-1][2] == r_lo:
                    merged[-1] = (h, merged[-1][1], r_hi)
                else:
                    merged.append((h, r_lo, r_hi))
            runs = merged
        plan.sgs.append((t_lo, len(plan.tiles), runs, f_lo, f_hi))
        f_lo = f_hi
    T = len(plan.tiles)

    for c in range(C):
        idx_flat = np.zeros(T * 128, np.int32)
        dr = np.full((128, T), -1.0, np.float16)
        cursor = [[0, 0] for _ in range(n_frames)]
        for t, (f, h) in enumerate(plan.tiles):
            s_arr, dl_arr = groups[c][f][h]
            lo = cursor[f][h]
            take = max(0, min(128, len(s_arr) - lo))
            cursor[f][h] = lo + take
            if take > 0:
                s_t = s_arr[lo:lo + take]
                w_t = (dl_arr[lo:lo + take] - f * F).astype(np.int64)
                assert w_t.min() >= 0 and w_t.max() < F
                idx_flat[t * 128:t * 128 + take] = s_t
                dr[:take, t] = w_t
        idxw = np.zeros((128, T * 8), np.int16)
        block = idx_flat.reshape(T * 8, 16).T.astype(np.int16)
        for g in range(8):
            idxw[16 * g:16 * g + 16] = block
        plan.idx16.append(idxw)
        plan.dstrel.append(dr)
        # valid (unpadded) index count per gather run, rounded up to 16
        gc = np.array([min((len(groups[c][f][h][0]) + 15) // 16 * 16,
                           128 * sum(1 for t in plan.tiles if t == (f, h)))
                       for (f, h) in plan.run_group], np.int32)
        plan.gcnt.append(gc.reshape(1, -1))
    return plan


def build_pool_mats(cfg: GATConfig, batch: np.ndarray) -> list:
    """Per-core [frame, n_frames] f32 batch-id-per-slot tables (one-hot built
    on-chip via is_equal against an iota over graph ids)."""
    out = []
    NL, F, NF = cfg.nloc, cfg.frame, cfg.n_frames
    for c in range(cfg.n_cores):
        b = batch[c * NL:(c + 1) * NL].astype(np.float32)
        padded = np.zeros(NF * F, np.float32)
        padded[:NL] = b
        out.append(np.ascontiguousarray(padded.reshape(NF, F).T))
    return out


def build_bass(cfg: GATConfig, plan: EdgePlan, timing: bool = False,
               phases: frozenset = frozenset({"build", "edge", "pool"}),
               skip: frozenset = frozenset(), reps: int = 1):
    C, NL, F, D, G = cfg.n_cores, cfg.nloc, cfg.frame, cfg.dim, cfg.n_graphs
    NT, NLP, RL, T = cfg.node_tiles, cfg.nloc_pad, cfg.rowlen, len(plan.tiles)
    NF = cfg.n_frames
    SI = cfg.split_i
    NTA = SI // 128
    TB = 132  # trow fp16 cols: h(128) | s | one | pad2

    nc = bacc.Bacc("TRN2", target_bir_lowering=False, debug=False,
                   num_devices=1 if timing else C, num_swdge_queues=4)

    xT_in = nc.dram_tensor("xT", [D, NLP], FP16, kind="ExternalInput").ap()
    wt = [nc.dram_tensor(f"wt{l}", [D, D], FP16, kind="ExternalInput").ap() for l in range(3)]
    ad = [nc.dram_tensor(f"ad{l}", [D, 2], FP16, kind="ExternalInput").ap() for l in range(3)]
    idx_in = nc.dram_tensor("idx", [128, T * 8], I16, kind="ExternalInput").ap()
    dr_in = nc.dram_tensor("dstrel", [128, T], FP16, kind="ExternalInput").ap()
    NRUNS = len(plan.run_group)
    if cfg.trim_pads:
        gcnt_in = nc.dram_tensor("gcnt", [1, NRUNS], mybir.dt.int32,
                                 kind="ExternalInput").ap()
    b_in = nc.dram_tensor("bslot", [cfg.frame, NF], FP32, kind="ExternalInput").ap()
    out_ext = nc.dram_tensor("out", [G, D], FP32, kind="ExternalOutput").ap()

    with tile.TileContext(nc) as tc:
        with tc.tile_pool(name="const", bufs=1) as cpool, \
             tc.tile_pool(name="master", bufs=1) as mpool, \
             tc.tile_pool(name="build", bufs=2) as bpool, \
             tc.tile_pool(name="bpsum", bufs=2, space="PSUM") as bps, \
             tc.tile_pool(name="edge", bufs=2) as epool, \
             tc.tile_pool(name="escal", bufs=3) as spool, \
             tc.tile_pool(name="fpsum", bufs=2, space="PSUM") as fps, \
             tc.tile_pool(name="dram", bufs=1, space="DRAM") as dr:

            ident = cpool.tile([128, 128], FP32)
            make_identity(nc, ident[:])
            ident16 = cpool.tile([128, 128], FP16)
            nc.vector.tensor_copy(out=ident16[:], in_=ident[:])

            # resident masters
            xT = mpool.tile([128, NLP], FP16)        # features^T (dim x node)
            x64 = mpool.tile([F, NF, D], FP16)      # features, frame-major
            dmast = mpool.tile([128, NLP], FP16)     # d[n] broadcast to all parts
            idx_sb = mpool.tile([128, T * 8], I16)
            drel_sb = mpool.tile([128, T], FP16)
            SGMAX = max(t_hi - t_lo for (t_lo, t_hi, _, _, _) in plan.sgs)
            iota64 = cpool.tile([128, F, SGMAX], FP16)
            nc.gpsimd.iota(iota64[:], pattern=[[1, F], [0, SGMAX]], base=0,
                           channel_multiplier=0,
                           allow_small_or_imprecise_dtypes=True)

            nc.sync.dma_start(out=xT[:], in_=xT_in[:, :])
            nc.sync.dma_start(out=idx_sb[:], in_=idx_in[:, :])
            nc.sync.dma_start(out=drel_sb[:], in_=dr_in[:, :])
            if cfg.trim_pads:
                gcnt_sb = mpool.tile([1, NRUNS], mybir.dt.int32)
                nc.sync.dma_start(out=gcnt_sb[:], in_=gcnt_in[:, :])
                gcnt_regs = [nc.gpsimd.alloc_register(f"gcnt_reg{q}")
                             for q in range(4)]

            bslot_sb = cpool.tile([F, NF], FP32)
            nc.sync.dma_start(out=bslot_sb[:], in_=b_in[:, :])
            iota_g = cpool.tile([F, G], FP32)
            nc.gpsimd.iota(iota_g[:], pattern=[[1, G]], base=0,
                           channel_multiplier=0,
                           allow_small_or_imprecise_dtypes=True)

            # zero the rotating gather buffers once: pad tile slots gather row
            # 0 (finite), but keep the unwritten tails deterministic anyway.
            for _b in range(cfg.g16_bufs):
                gz = epool.tile([128, SGMAX, RL], FP8, tag="g16", bufs=cfg.g16_bufs)
                nc.vector.memset(gz[:], 0.0)

            wt_sb = [cpool.tile([128, D], FP16, name=f"wt_sb{_l}") for _l in range(3)]
            ad_sb = [cpool.tile([128, 2], FP16, name=f"ad_sb{_l}") for _l in range(3)]
            for l in range(3):
                nc.sync.dma_start(out=wt_sb[l][:], in_=wt[l][:, :])
                nc.sync.dma_start(out=ad_sb[l][:], in_=ad[l][:, :])

            if NF * F > NL:
                nc.vector.memset(x64[:, NF - 1, :], 0.0)

            for rep in range(reps):
              for l in range(3):
                  # ---------- table build ----------
                  # trow tiles accumulate in an SBUF staging buffer; one SWDGE
                  # DMA per half casts fp16 -> fp8 into the 256B-row shard.
                  tight_a = dr.tile([cfg.rows_a, RL], FP8,
                                    addr_space="Local" if timing else "Shared",
                                    tag="tight_a", name=f"tight_a{l}_{rep}",
                                    bufs=2)
                  tight_b = dr.tile([cfg.rows_b, RL], FP8,
                                    addr_space="Local" if timing else "Shared",
                                    tag="tight_b", name=f"tight_b{l}_{rep}",
                                    bufs=2)
                  shard_a = dr.tile([SI, RL], FP8, tag="shard_a",
                                    name=f"shard_a{l}_{rep}", bufs=2)
                  shard_b = dr.tile([NL - SI, RL], FP8, tag="shard_b",
                                    name=f"shard_b{l}_{rep}", bufs=2)
                  trowbuf = bpool.tile([128, NT, TB], FP16, tag="trowbuf")

                  def emit_ag(shard, tight, nsh):
                      if timing or "ag" in skip:
                          if "build" in phases:
                              nc.sync.dma_start(out=tight[0:nsh, :], in_=shard[:, :])
                      else:
                          nc.gpsimd.collective_compute(
                              "AllGather", mybir.AluOpType.bypass,
                              replica_groups=[list(range(C))],
                              ins=[shard.opt()], outs=[tight.opt()],
                          )

                  def emit_shard_dma(t0, t1, shard, nrows):
                      # shard viewed [part, tile, col] to match the SBUF
                      # iteration order; SWDGE casts fp16 -> fp8.  The last
                      # tile may be partial (node padding) - split it off.
                      nfull = (t1 - t0) if (t1 - t0) * 128 == nrows else (t1 - t0 - 1)
                      if nfull > 0:
                          shard_v = bass.AP(
                              shard.tensor, shard.offset,
                              [[RL, 128], [128 * RL, nfull], [1, 130]])
                          nc.gpsimd.dma_start(
                              out=shard_v, in_=trowbuf[:, t0:t0 + nfull, 0:130])
                      if nfull < t1 - t0:
                          tail = nrows - nfull * 128
                          shard_t = bass.AP(
                              shard.tensor, shard.offset + nfull * 128 * RL,
                              [[RL, tail], [1, 130]])
                          nc.gpsimd.dma_start(
                              out=shard_t,
                              in_=trowbuf[0:tail, t1 - 1, 0:130])

                  if "build" in phases:
                      nc.vector.memset(trowbuf[:, :, 129:130], 1.0)
                  fpt = 128 // F
                  for t in range(NT) if "build" in phases else []:
                      if l > 0:
                          for f in range(t * fpt, min((t + 1) * fpt, NF)):
                              tps = bps.tile([128, F], FP16, space="PSUM",
                                             tag="bps")
                              nc.tensor.transpose(out=tps[:], in_=x64[:, f, :],
                                                  identity=ident16[0:F, 0:F])
                              nc.vector.tensor_copy(out=xT[:, f * F:(f + 1) * F],
                                                    in_=tps[:])
                      hps = bps.tile([128, 130], FP32, space="PSUM", tag="bps")
                      lhsT = xT[:, t * 128:(t + 1) * 128]
                      nc.tensor.matmul(out=hps[:, 0:128], lhsT=lhsT, rhs=wt_sb[l][:],
                                       start=True, stop=True)
                      nc.tensor.matmul(out=hps[:, 128:130], lhsT=lhsT, rhs=ad_sb[l][:],
                                       start=True, stop=True)
                      # h(128) | s -> trow cols 0:129 in one copy
                      nc.vector.tensor_copy(out=trowbuf[:, t, 0:129],
                                            in_=hps[:, 0:129])
                      if t == NTA - 1:
                          emit_shard_dma(0, NTA, shard_a, SI)
                          emit_ag(shard_a, tight_a, SI)
                      elif t == NT - 1:
                          emit_shard_dma(NTA, NT, shard_b, NL - SI)
                          emit_ag(shard_b, tight_b, NL - SI)
                  for ch in range(0, NLP, 512) if "build" in phases else []:
                      cw = min(512, NLP - ch)
                      dps = bps.tile([128, 512], FP32, space="PSUM", tag="bps")
                      nc.tensor.matmul(
                          out=dps[:, :cw],
                          lhsT=ad_sb[l][:, 1:2].to_broadcast([128, 128]),
                          rhs=xT[:, ch:ch + cw], start=True, stop=True)
                      nc.vector.tensor_copy(out=dmast[:, ch:ch + cw],
                                            in_=dps[:, :cw])

                  # ---------- edge phase ----------
                  run_id = 0
                  for (t_lo, t_hi, runs, f_lo, f_hi) in (plan.sgs if "edge" in phases else []):
                      nt = t_hi - t_lo
                      g16 = epool.tile([128, nt, RL], FP8, tag="g16", bufs=cfg.g16_bufs)
                      if "gather" in skip:
                          nc.vector.memset(g16[:, 0, 0:1], 0.0)
                      for ri, (h, r_lo, r_hi) in enumerate(runs):
                          if "gather" in skip:
                              continue
                          nidx = (r_hi - r_lo) * 128
                          if cfg.trim_pads:
                              nreg = gcnt_regs[(t_lo + ri) % 4]
                              nc.gpsimd.reg_load(
                                  nreg,
                                  gcnt_sb[0:1, run_id + ri:run_id + ri + 1])
                          else:
                              nreg = nidx
                          src_ap = tight_b[:, :] if h else tight_a[:, :]
                          nc.gpsimd.dma_gather(
                              out_ap=g16[:, r_lo - t_lo:r_hi - t_lo, :],
                              in_ap=src_ap,
                              idxs_ap=idx_sb[:, r_lo * 8:r_hi * 8],
                              num_idxs=nidx, num_idxs_reg=nreg, elem_size=RL,
                              single_packet=cfg.single_packet,
                              queue_num=(t_lo + ri) % 4,
                          )
                      run_id += len(runs)
                      p16 = epool.tile([128, F, nt], FP16, tag="p16")
                      drel_ap = bass.AP(drel_sb.tensor, drel_sb.offset + t_lo,
                                        [drel_sb.ap[0], [0, F], [1, nt]])
                      iota_ap = bass.AP(iota64.tensor, iota64.offset,
                                        [iota64.ap[0], [SGMAX, F], [1, nt]])
                      if "eq" not in skip:
                          nc.vector.tensor_tensor(out=p16[:], in0=drel_ap, in1=iota_ap,
                                                  op=mybir.AluOpType.is_equal)
                      else:
                          nc.vector.memset(p16[:, 0, 0:1], 0.0)
                      pd = epool.tile([128, F, nt], FP16, tag="pd")
                      if "pd" in skip:
                          nc.vector.memset(pd[:, 0, 0:1], 0.0)
                      for f in (range(f_lo, f_hi) if "pd" not in skip else []):
                          for (t0, fnt) in plan.frame_spans.get(f, []):
                              ft0 = t0 - t_lo
                              dm_ap = bass.AP(dmast.tensor, dmast.offset + f * F,
                                              [dmast.ap[0], [1, F], [0, fnt]])
                              nc.vector.tensor_tensor(
                                  out=pd[:, :, ft0:ft0 + fnt],
                                  in0=p16[:, :, ft0:ft0 + fnt],
                                  in1=dm_ap,
                                  op=mybir.AluOpType.mult)
                      dx = spool.tile([128, nt], FP16, tag="dx")
                      if "dx" in skip:
                          nc.vector.memset(dx[:], 0.0)
                      else:
                          pd_t = bass.AP(pd.tensor, pd.offset,
                                         [pd.ap[0], [1, nt], [nt, F]])
                          with nc.allow_low_precision(
                                  reason="one-hot masked sum: exactly one nonzero term"):
                              nc.vector.tensor_reduce(out=dx[:], in_=pd_t,
                                                      axis=mybir.AxisListType.X,
                                                      op=mybir.AluOpType.add)
                      eL = spool.tile([128, nt], FP16, tag="eL")
                      s_ap = bass.AP(g16.tensor, g16.offset + 128,
                                     [g16.ap[0], [RL, nt]])
                      nc.vector.tensor_tensor(out=eL[:], in0=s_ap, in1=dx[:],
                                              op=mybir.AluOpType.add)
                      ee = spool.tile([128, nt], FP16, tag="ee")
                      if "act" in skip:
                          nc.vector.memset(ee[:], 1.0)
                      else:
                          lr = spool.tile([128, nt], FP16, tag="lr")
                          nc.vector.scalar_tensor_tensor(
                              out=lr[:], in0=eL[:], scalar=NEG_SLOPE_ATT,
                              in1=eL[:], op0=mybir.AluOpType.mult,
                              op1=mybir.AluOpType.max)
                          nc.scalar.activation(out=ee[:], in_=lr[:],
                                               func=mybir.ActivationFunctionType.Exp)
                      pw = epool.tile([128, F, nt], FP16, tag="pw")
                      if "pw" not in skip:
                          ee_ap = bass.AP(ee.tensor, ee.offset,
                                          [ee.ap[0], [0, F], [1, nt]])
                          nc.vector.tensor_tensor(out=pw[:], in0=p16[:],
                                                  in1=ee_ap,
                                                  op=mybir.AluOpType.mult)
                      else:
                          nc.vector.memset(pw[:, 0, 0:1], 0.0)

                      for f in range(f_lo, f_hi):
                          spans = plan.frame_spans.get(f, [])
                          fnt = sum(n for (_, n) in spans)
                          if fnt == 0:
                              continue
                          acc = fps.tile([F, 130], FP32, space="PSUM", tag="acc",
                                         bufs=4)
                          if "scmm" in skip:
                              nc.vector.memset(acc[:, 0:1], 0.0)
                          else:
                              k = 0
                              for (t0, n) in spans:
                                  for j in range(n):
                                      ft = t0 - t_lo + j
                                      pw_j = bass.AP(pw.tensor, pw.offset + ft,
                                                     [pw.ap[0], [nt, F]])
                                      nc.tensor.matmul(
                                          out=acc[:], lhsT=pw_j,
                                          rhs=g16[:, ft, 0:130],
                                          start=(k == 0), stop=(k == fnt - 1))
                                      k += 1
                          nw = NL - f * F if f == NF - 1 and NF * F > NL else F
                          xslice = x64[0:nw, f, :]
                          if "norm" in skip:
                              nc.vector.tensor_copy(out=xslice, in_=acc[:nw, 0:128])
                              continue
                          zr = spool.tile([F, 1], FP32, tag="zr")
                          nc.vector.reciprocal(out=zr[:nw], in_=acc[:nw, 129:130])
                          if l == 1:
                              xr = spool.tile([F, D], FP32, tag="xr")
                              nc.vector.tensor_scalar(xr[:nw], acc[:nw, 0:128], zr[:nw],
                                                      None, mybir.AluOpType.mult)
                              nc.vector.scalar_tensor_tensor(
                                  out=xslice, in0=xr[:nw], scalar=NEG_SLOPE_ACT,
                                  in1=xr[:nw], op0=mybir.AluOpType.mult,
                                  op1=mybir.AluOpType.max)
                          else:
                              nc.vector.tensor_scalar(xslice, acc[:nw, 0:128], zr[:nw],
                                                      None, mybir.AluOpType.mult)

            # ---------- pooling ----------
            pool_ps = fps.tile([128, G], FP32, space="PSUM", tag="pool", bufs=1)
            for f in range(NF) if "pool" in phases else []:
                bt = bpool.tile([F, G], FP16, tag="bt")
                b_col = bass.AP(bslot_sb.tensor, bslot_sb.offset + f,
                                [bslot_sb.ap[0], [0, G]])
                nc.vector.tensor_tensor(out=bt[:], in0=b_col, in1=iota_g[:],
                                        op=mybir.AluOpType.is_equal)
                nc.tensor.matmul(out=pool_ps[:], lhsT=x64[:, f, :], rhs=bt[:],
                                 start=(f == 0), stop=(f == NF - 1))
            pooled_t = bpool.tile([128, G], FP32, tag="pooledt")  # [dim, graph]
            nc.vector.tensor_copy(out=pooled_t[:], in_=pool_ps[:])
            ar_in = dr.tile([128, G], FP32)
            ar_out = dr.tile([128, G], FP32,
                             addr_space="Local" if timing else "Shared")
            nc.sync.dma_start(out=ar_in[:], in_=pooled_t[:])
            if timing:
                nc.sync.dma_start(out=ar_out[:], in_=ar_in[:])
            else:
                nc.gpsimd.collective_compute(
                    "AllReduce", mybir.AluOpType.add,
                    replica_groups=[list(range(C))],
                    ins=[ar_in.opt()], outs=[ar_out.opt()],
                )
            red = bpool.tile([128, G], FP32, tag="red")
            nc.sync.dma_start(out=red[:], in_=ar_out[:])
            for gt in range(math.ceil(G / 128)):
                gcols = min(128, G - gt * 128)
                tps2 = bps.tile([128, 128], FP32, space="PSUM", tag="bps")
                nc.tensor.transpose(out=tps2[:gcols, :],
                                    in_=red[:, gt * 128:gt * 128 + gcols],
                                    identity=ident[:])
                og = bpool.tile([128, 128], FP32, tag="og")
                nc.vector.tensor_copy(out=og[:gcols, :], in_=tps2[:gcols, :])
                nc.sync.dma_start(out=out_ext[gt * 128:gt * 128 + gcols, :],
                                  in_=og[:gcols, :])
    nc.compile()
    return nc


_CACHE = {}


def _get_compiled(cfg: GATConfig, edge_index, batch):
    key = (cfg.n_nodes, cfg.n_edges, edge_index.tobytes()[:64], batch.tobytes()[:64])
    if key not in _CACHE:
        plan = build_edge_plan(cfg, edge_index)
        pools = build_pool_mats(cfg, batch)
        nc = build_bass(cfg, plan)
        _CACHE[key] = (nc, plan, pools)
    return _CACHE[key]


def build_in_maps(cfg: GATConfig, plan: EdgePlan, pools: list, inputs: dict):
    x = np.asarray(inputs["x"], np.float32)
    C, NL, NLP, D = cfg.n_cores, cfg.nloc, cfg.nloc_pad, cfg.dim
    in_maps = []
    for c in range(C):
        xs = np.zeros((D, NLP), np.float16)
        xs[:, :NL] = x[c * NL:(c + 1) * NL].T.astype(np.float16)
        m = {"xT": xs, "idx": plan.idx16[c], "dstrel": plan.dstrel[c],
             "bslot": pools[c], "gcnt": plan.gcnt[c]}
        for l in range(3):
            W = np.asarray(inputs[f"W{l}"], np.float32)
            a_s = np.asarray(inputs[f"a_src{l}"], np.float32)
            a_d = np.asarray(inputs[f"a_dst{l}"], np.float32)
            m[f"wt{l}"] = np.ascontiguousarray(W.T.astype(np.float16))
            m[f"ad{l}"] = np.ascontiguousarray(
                np.stack([W.T @ a_s, W.T @ a_d], axis=1).astype(np.float16))
        in_maps.append(m)
    return in_maps


def gat_forward(cfg: GATConfig, inputs: dict, trace: bool = False):
    edge_index = np.asarray(inputs["edge_index"])
    batch = np.asarray(inputs["batch"])
    nc, plan, pools = _get_compiled(cfg, edge_index, batch)
    in_maps = build_in_maps(cfg, plan, pools, inputs)
    r = run_bass_kernel_spmd(nc, in_maps, list(range(cfg.n_cores)), trace=trace)
    return (r.results[0]["out"], r) if trace else (r.results[0]["out"], None)


def kernel(**inputs) -> np.ndarray:
    cfg = GATConfig()
    out, _ = gat_forward(cfg, inputs)
    return out


# revision 53
# speedup vs baseline: 1.0383x; 1.0383x over previous
"""Distributed 3-layer GAT kernel for Trainium2 (8 NeuronCores), v2.

Sharding: nodes partitioned contiguously (N/8 per core); each edge is owned by
the core owning its dst node.  Per layer: each core computes the feature-table
rows for its own nodes (h = x @ W.T in fp16, the attention scalar s = h.a_src,
a constant-one z column), stages them in SBUF, bulk-DMAs them (with an
fp16->fp8 cast) into 256B fp8 shard rows, and AllGathers the table - no
re-padding pass, the AG output is gathered from directly.  Edges are then
processed in supergroups of whole 64-dst frames, tiles ordered half-major:
dma_gather of source rows (256B fp8, one call per (frame, half) group - the
empirically fastest call granularity), one-hot tile masks p16 = (dstrel ==
iota) in [edge, slot, tile] layout (dense inner strides for the DVE 2x mode),
d-lookup via pd = p16*d and a strided reduce, attention weights
ee = exp(leakyrelu(s_src + d_dst)) (leaky on DVE, exp on ACT - the only ACT
op, keeping its function table warm), and scatter-add via ee-scaled one-hot
matmuls (fp16 x fp8) into per-frame PSUM accumulators; out = num / z.
Global add-pool via one-hot matmul + AllReduce.  The kernel is bound by the
row-granular gather stream (~3.4 ns/row); everything else overlaps it.

All per-edge index structure (gather indices, dstrel, static tile schedule)
is host-precomputed from edge_index; the same instruction stream runs SPMD on
all 8 cores, per-core variation enters only through data.
"""
import math
from dataclasses import dataclass, field

import numpy as np

import concourse.bacc as bacc
import concourse.tile as tile
from concourse import bass, mybir
from concourse.bass_utils import run_bass_kernel_spmd
from concourse.masks import make_identity

FP32 = mybir.dt.float32
FP16 = mybir.dt.float16
FP8 = mybir.dt.float8e4
I16 = mybir.dt.int16

NEG_SLOPE_ATT = 0.2
NEG_SLOPE_ACT = 0.01


@dataclass
class GATConfig:
    n_nodes: int = 50000
    n_edges: int = 800000
    dim: int = 128
    n_graphs: int = 512
    n_cores: int = 8
    frame: int = 64             # dst window per scatter accumulation
    sg_tile_budget: int = 24    # max tiles per supergroup (gather batch)
    h_major: bool = True        # order sg tiles half-major (2 gathers per sg)
    single_packet: bool = False
    trim_pads: bool = False     # per-core num_idxs_reg from SBUF counts
    g16_bufs: int = 3
    big_runs: bool = False      # merge same-half gather runs across frames
    split_runs: int = 1         # split each gather run into this many calls

    @property
    def nloc(self):
        return self.n_nodes // self.n_cores

    @property
    def n_frames(self):
        return math.ceil(self.nloc / self.frame)

    @property
    def node_tiles(self):
        return math.ceil(self.nloc / 128)

    @property
    def nloc_pad(self):
        return self.node_tiles * 128

    @property
    def split_i(self):
        # tile-aligned split of each core's shard into A/B halves; keeps both
        # gather tables under the int16 index limit of dma_gather
        return (self.nloc // 2 // 128) * 128

    @property
    def rows_a(self):
        return self.n_cores * self.split_i

    @property
    def rows_b(self):
        return self.n_cores * (self.nloc - self.split_i)

    @property
    def rowlen(self):
        # fp8 elems per table row: h(128) | s | one | pad; 256B rows satisfy
        # the dma_gather 256B-multiple constraint with no re-pad pass
        return 256


@dataclass
class EdgePlan:
    tiles: list = field(default_factory=list)            # per tile: (frame, half)
    frame_spans: dict = field(default_factory=dict)      # f -> [(t0, n), ...]
    sgs: list = field(default_factory=list)              # (t_lo, t_hi, runs, f_lo, f_hi)
    run_group: list = field(default_factory=list)        # per run: (f, h)
    idx16: list = field(default_factory=list)            # per core [128, T*8] int16
    dstrel: list = field(default_factory=list)           # per core [128, T] fp16
    gcnt: list = field(default_factory=list)             # per core [1, nruns] int32


def build_edge_plan(cfg: GATConfig, edge_index: np.ndarray) -> EdgePlan:
    N, C, NL, F = cfg.n_nodes, cfg.n_cores, cfg.nloc, cfg.frame
    src = edge_index[0].astype(np.int64)
    dst = edge_index[1].astype(np.int64)
    loops = np.arange(N, dtype=np.int64)
    src = np.concatenate([src, loops])
    dst = np.concatenate([dst, loops])

    n_frames = cfg.n_frames
    SI = cfg.split_i
    src_core = src // NL
    src_i = src - src_core * NL
    src_hi = (src_i >= SI).astype(np.int64)
    gidx = np.where(src_hi == 0, src_core * SI + src_i,
                    src_core * (NL - SI) + (src_i - SI))
    assert gidx.max() < 32768
    groups = [[[None, None] for _ in range(n_frames)] for _ in range(C)]
    core_of = dst // NL
    dst_loc = dst - core_of * NL
    frame_of = dst_loc // F
    for c in range(C):
        m = core_of == c
        s_c, dl_c, f_c = gidx[m], dst_loc[m], frame_of[m]
        hi_c = src_hi[m]
        order = np.lexsort((dl_c, hi_c, f_c))
        s_c, dl_c, f_c, hi_c = s_c[order], dl_c[order], f_c[order], hi_c[order]
        key = f_c * 2 + hi_c
        bounds = np.searchsorted(key, np.arange(2 * n_frames + 1))
        for f in range(n_frames):
            for h in range(2):
                lo, hi = bounds[2 * f + h], bounds[2 * f + h + 1]
                groups[c][f][h] = (s_c[lo:hi], dl_c[lo:hi])

    plan = EdgePlan()
    # per-(frame, half) tile counts (max across cores)
    ntiles = [[0, 0] for _ in range(n_frames)]
    for f in range(n_frames):
        for h in range(2):
            cnt = max(len(groups[c][f][h][0]) for c in range(C))
            ntiles[f][h] = math.ceil(cnt / 128)

    # supergroups span whole frames; tiles within an sg are ordered h-major
    # (all half-0 tiles first, then half-1) so each sg needs only TWO
    # dma_gather calls - the SWDGE per-call overhead dominates otherwise.
    f_lo = 0
    while f_lo < n_frames:
        f_hi = f_lo
        tot = 0
        while f_hi < n_frames and tot + sum(ntiles[f_hi]) <= cfg.sg_tile_budget:
            tot += sum(ntiles[f_hi])
            f_hi += 1
        assert f_hi > f_lo, "single frame exceeds sg_tile_budget"
        t_lo = len(plan.tiles)
        order = ([(h, f) for h in range(2) for f in range(f_lo, f_hi)]
                 if cfg.h_major else
                 [(h, f) for f in range(f_lo, f_hi) for h in range(2)])
        # one gather run per (frame, half) group: each core's pad slots form a
        # contiguous tail of the group, so -1 indices there are skipped by
        # dma_gather ("negative indices at the end are ignored")
        runs = []
        for (h, f) in order:
            t0 = len(plan.tiles)
            for _ in range(ntiles[f][h]):
                plan.tiles.append((f, h))
            if ntiles[f][h]:
                spans = plan.frame_spans.setdefault(f, [])
                if spans and spans[-1][0] + spans[-1][1] == t0:
                    spans[-1] = (spans[-1][0], spans[-1][1] + ntiles[f][h])
                else:
                    spans.append((t0, ntiles[f][h]))
                runs.append((h, t0, len(plan.tiles)))
                plan.run_group.append((f, h))
        if cfg.big_runs:
            merged = []
            for (h, r_lo, r_hi) in runs:
                if merged and merged[-1][0] == h and merged[-1][2] == r_lo:
                    merged[-1] = (h, merged[-1][1], r_hi)
                else:
                    merged.append((h, r_lo, r_hi))
            runs = merged
        if cfg.split_runs > 1:
            split = []
            for (h, r_lo, r_hi) in runs:
                n = r_hi - r_lo
                k = min(cfg.split_runs, n)
                for i in range(k):
                    a = r_lo + n * i // k
                    b = r_lo + n * (i + 1) // k
                    if b > a:
                        split.append((h, a, b))
            runs = split
        plan.sgs.append((t_lo, len(plan.tiles), runs, f_lo, f_hi))
        f_lo = f_hi
    T = len(plan.tiles)

    for c in range(C):
        idx_flat = np.zeros(T * 128, np.int32)
        dr = np.full((128, T), -1.0, np.float16)
        cursor = [[0, 0] for _ in range(n_frames)]
        for t, (f, h) in enumerate(plan.tiles):
            s_arr, dl_arr = groups[c][f][h]
            lo = cursor[f][h]
            take = max(0, min(128, len(s_arr) - lo))
            cursor[f][h] = lo + take
            if take > 0:
                s_t = s_arr[lo:lo + take]
                w_t = (dl_arr[lo:lo + take] - f * F).astype(np.int64)
                assert w_t.min() >= 0 and w_t.max() < F
                idx_flat[t * 128:t * 128 + take] = s_t
                dr[:take, t] = w_t
        idxw = np.zeros((128, T * 8), np.int16)
        block = idx_flat.reshape(T * 8, 16).T.astype(np.int16)
        for g in range(8):
            idxw[16 * g:16 * g + 16] = block
        plan.idx16.append(idxw)
        plan.dstrel.append(dr)
        # valid (unpadded) index count per gather run, rounded up to 16
        gc = np.array([min((len(groups[c][f][h][0]) + 15) // 16 * 16,
                           128 * sum(1 for t in plan.tiles if t == (f, h)))
                       for (f, h) in plan.run_group], np.int32)
        plan.gcnt.append(gc.reshape(1, -1))
    return plan


def build_pool_mats(cfg: GATConfig, batch: np.ndarray) -> list:
    """Per-core [frame, n_frames] f32 batch-id-per-slot tables (one-hot built
    on-chip via is_equal against an iota over graph ids)."""
    out = []
    NL, F, NF = cfg.nloc, cfg.frame, cfg.n_frames
    for c in range(cfg.n_cores):
        b = batch[c * NL:(c + 1) * NL].astype(np.float32)
        padded = np.zeros(NF * F, np.float32)
        padded[:NL] = b
        out.append(np.ascontiguousarray(padded.reshape(NF, F).T))
    return out


def build_bass(cfg: GATConfig, plan: EdgePlan, timing: bool = False,
               phases: frozenset = frozenset({"build", "edge", "pool"}),
               skip: frozenset = frozenset(), reps: int = 1):
    C, NL, F, D, G = cfg.n_cores, cfg.nloc, cfg.frame, cfg.dim, cfg.n_graphs
    NT, NLP, RL, T = cfg.node_tiles, cfg.nloc_pad, cfg.rowlen, len(plan.tiles)
    NF = cfg.n_frames
    SI = cfg.split_i
    NTA = SI // 128
    TB = 132  # trow fp16 cols: h(128) | s | one | pad2

    nc = bacc.Bacc("TRN2", target_bir_lowering=False, debug=False,
                   num_devices=1 if timing else C, num_swdge_queues=4)

    xT_in = nc.dram_tensor("xT", [D, NLP], FP16, kind="ExternalInput").ap()
    wt = [nc.dram_tensor(f"wt{l}", [D, D], FP16, kind="ExternalInput").ap() for l in range(3)]
    ad = [nc.dram_tensor(f"ad{l}", [D, 2], FP16, kind="ExternalInput").ap() for l in range(3)]
    idx_in = nc.dram_tensor("idx", [128, T * 8], I16, kind="ExternalInput").ap()
    dr_in = nc.dram_tensor("dstrel", [128, T], FP16, kind="ExternalInput").ap()
    NRUNS = len(plan.run_group)
    if cfg.trim_pads:
        gcnt_in = nc.dram_tensor("gcnt", [1, NRUNS], mybir.dt.int32,
                                 kind="ExternalInput").ap()
    b_in = nc.dram_tensor("bslot", [cfg.frame, NF], FP32, kind="ExternalInput").ap()
    out_ext = nc.dram_tensor("out", [G, D], FP32, kind="ExternalOutput").ap()

    with tile.TileContext(nc) as tc:
        with tc.tile_pool(name="const", bufs=1) as cpool, \
             tc.tile_pool(name="master", bufs=1) as mpool, \
             tc.tile_pool(name="build", bufs=2) as bpool, \
             tc.tile_pool(name="bpsum", bufs=2, space="PSUM") as bps, \
             tc.tile_pool(name="edge", bufs=2) as epool, \
             tc.tile_pool(name="escal", bufs=3) as spool, \
             tc.tile_pool(name="fpsum", bufs=2, space="PSUM") as fps, \
             tc.tile_pool(name="dram", bufs=1, space="DRAM") as dr:

            ident = cpool.tile([128, 128], FP32)
            make_identity(nc, ident[:])
            ident16 = cpool.tile([128, 128], FP16)
            nc.vector.tensor_copy(out=ident16[:], in_=ident[:])

            # resident masters
            xT = mpool.tile([128, NLP], FP16)        # features^T (dim x node)
            x64 = mpool.tile([F, NF, D], FP16)      # features, frame-major
            dmast = mpool.tile([128, NLP], FP16)     # d[n] broadcast to all parts
            idx_sb = mpool.tile([128, T * 8], I16)
            drel_sb = mpool.tile([128, T], FP16)
            SGMAX = max(t_hi - t_lo for (t_lo, t_hi, _, _, _) in plan.sgs)
            iota64 = cpool.tile([128, F, SGMAX], FP16)
            nc.gpsimd.iota(iota64[:], pattern=[[1, F], [0, SGMAX]], base=0,
                           channel_multiplier=0,
                           allow_small_or_imprecise_dtypes=True)

            nc.sync.dma_start(out=xT[:], in_=xT_in[:, :])
            nc.sync.dma_start(out=idx_sb[:], in_=idx_in[:, :])
            nc.sync.dma_start(out=drel_sb[:], in_=dr_in[:, :])
            if cfg.trim_pads:
                gcnt_sb = mpool.tile([1, NRUNS], mybir.dt.int32)
                nc.sync.dma_start(out=gcnt_sb[:], in_=gcnt_in[:, :])
                gcnt_regs = [nc.gpsimd.alloc_register(f"gcnt_reg{q}")
                             for q in range(4)]

            bslot_sb = cpool.tile([F, NF], FP32)
            nc.sync.dma_start(out=bslot_sb[:], in_=b_in[:, :])
            iota_g = cpool.tile([F, G], FP32)
            nc.gpsimd.iota(iota_g[:], pattern=[[1, G]], base=0,
                           channel_multiplier=0,
                           allow_small_or_imprecise_dtypes=True)

            # zero the rotating gather buffers once: pad tile slots gather row
            # 0 (finite), but keep the unwritten tails deterministic anyway.
            for _b in range(cfg.g16_bufs):
                gz = epool.tile([128, SGMAX, RL], FP8, tag="g16", bufs=cfg.g16_bufs)
                nc.vector.memset(gz[:], 0.0)

            wt_sb = [cpool.tile([128, D], FP16, name=f"wt_sb{_l}") for _l in range(3)]
            ad_sb = [cpool.tile([128, 2], FP16, name=f"ad_sb{_l}") for _l in range(3)]
            for l in range(3):
                nc.sync.dma_start(out=wt_sb[l][:], in_=wt[l][:, :])
                nc.sync.dma_start(out=ad_sb[l][:], in_=ad[l][:, :])

            if NF * F > NL:
                nc.vector.memset(x64[:, NF - 1, :], 0.0)

            for rep in range(reps):
              for l in range(3):
                  # ---------- table build ----------
                  # trow tiles accumulate in an SBUF staging buffer; one SWDGE
                  # DMA per half casts fp16 -> fp8 into the 256B-row shard.
                  tight_a = dr.tile([cfg.rows_a, RL], FP8,
                                    addr_space="Local" if timing else "Shared",
                                    tag="tight_a", name=f"tight_a{l}_{rep}",
                                    bufs=2)
                  tight_b = dr.tile([cfg.rows_b, RL], FP8,
                                    addr_space="Local" if timing else "Shared",
                                    tag="tight_b", name=f"tight_b{l}_{rep}",
                                    bufs=2)
                  shard_a = dr.tile([SI, RL], FP8, tag="shard_a",
                                    name=f"shard_a{l}_{rep}", bufs=2)
                  shard_b = dr.tile([NL - SI, RL], FP8, tag="shard_b",
                                    name=f"shard_b{l}_{rep}", bufs=2)
                  trowbuf = bpool.tile([128, NT, TB], FP16, tag="trowbuf")

                  def emit_ag(shard, tight, nsh):
                      if timing or "ag" in skip:
                          if "build" in phases:
                              nc.sync.dma_start(out=tight[0:nsh, :], in_=shard[:, :])
                      else:
                          nc.gpsimd.collective_compute(
                              "AllGather", mybir.AluOpType.bypass,
                              replica_groups=[list(range(C))],
                              ins=[shard.opt()], outs=[tight.opt()],
                          )

                  def emit_shard_dma(t0, t1, shard, nrows):
                      # shard viewed [part, tile, col] to match the SBUF
                      # iteration order; SWDGE casts fp16 -> fp8.  The last
                      # tile may be partial (node padding) - split it off.
                      nfull = (t1 - t0) if (t1 - t0) * 128 == nrows else (t1 - t0 - 1)
                      if nfull > 0:
                          shard_v = bass.AP(
                              shard.tensor, shard.offset,
                              [[RL, 128], [128 * RL, nfull], [1, 130]])
                          nc.gpsimd.dma_start(
                              out=shard_v, in_=trowbuf[:, t0:t0 + nfull, 0:130])
                      if nfull < t1 - t0:
                          tail = nrows - nfull * 128
                          shard_t = bass.AP(
                              shard.tensor, shard.offset + nfull * 128 * RL,
                              [[RL, tail], [1, 130]])
                          nc.gpsimd.dma_start(
                              out=shard_t,
                              in_=trowbuf[0:tail, t1 - 1, 0:130])

                  if "build" in phases:
                      nc.vector.memset(trowbuf[:, :, 129:130], 1.0)
                  fpt = 128 // F
                  for t in range(NT) if "build" in phases else []:
                      if l > 0:
                          for f in range(t * fpt, min((t + 1) * fpt, NF)):
                              tps = bps.tile([128, F], FP16, space="PSUM",
                                             tag="bps")
                              nc.tensor.transpose(out=tps[:], in_=x64[:, f, :],
                                                  identity=ident16[0:F, 0:F])
                              nc.vector.tensor_copy(out=xT[:, f * F:(f + 1) * F],
                                                    in_=tps[:])
                      hps = bps.tile([128, 130], FP32, space="PSUM", tag="bps")
                      lhsT = xT[:, t * 128:(t + 1) * 128]
                      nc.tensor.matmul(out=hps[:, 0:128], lhsT=lhsT, rhs=wt_sb[l][:],
                                       start=True, stop=True)
                      nc.tensor.matmul(out=hps[:, 128:130], lhsT=lhsT, rhs=ad_sb[l][:],
                                       start=True, stop=True)
                      # h(128) | s -> trow cols 0:129 in one copy
                      nc.vector.tensor_copy(out=trowbuf[:, t, 0:129],
                                            in_=hps[:, 0:129])
                      if t == NTA - 1:
                          emit_shard_dma(0, NTA, shard_a, SI)
                          emit_ag(shard_a, tight_a, SI)
                      elif t == NT - 1:
                          emit_shard_dma(NTA, NT, shard_b, NL - SI)
                          emit_ag(shard_b, tight_b, NL - SI)
                  for ch in range(0, NLP, 512) if "build" in phases else []:
                      cw = min(512, NLP - ch)
                      dps = bps.tile([128, 512], FP32, space="PSUM", tag="bps")
                      nc.tensor.matmul(
                          out=dps[:, :cw],
                          lhsT=ad_sb[l][:, 1:2].to_broadcast([128, 128]),
                          rhs=xT[:, ch:ch + cw], start=True, stop=True)
                      nc.vector.tensor_copy(out=dmast[:, ch:ch + cw],
                                            in_=dps[:, :cw])

                  # ---------- edge phase ----------
                  run_id = 0
                  for (t_lo, t_hi, runs, f_lo, f_hi) in (plan.sgs if "edge" in phases else []):
                      nt = t_hi - t_lo
                      g16 = epool.tile([128, nt, RL], FP8, tag="g16", bufs=cfg.g16_bufs)
                      if "gather" in skip:
                          nc.vector.memset(g16[:, 0, 0:1], 0.0)
                      for ri, (h, r_lo, r_hi) in enumerate(runs):
                          if "gather" in skip:
                              continue
                          nidx = (r_hi - r_lo) * 128
                          if cfg.trim_pads:
                              nreg = gcnt_regs[(t_lo + ri) % 4]
                              nc.gpsimd.reg_load(
                                  nreg,
                                  gcnt_sb[0:1, run_id + ri:run_id + ri + 1])
                          else:
                              nreg = nidx
                          src_ap = tight_b[:, :] if h else tight_a[:, :]
                          nc.gpsimd.dma_gather(
                              out_ap=g16[:, r_lo - t_lo:r_hi - t_lo, :],
                              in_ap=src_ap,
                              idxs_ap=idx_sb[:, r_lo * 8:r_hi * 8],
                              num_idxs=nidx, num_idxs_reg=nreg, elem_size=RL,
                              single_packet=cfg.single_packet,
                              queue_num=(t_lo + ri) % 4,
                          )
                      run_id += len(runs)
                      p16 = epool.tile([128, F, nt], FP16, tag="p16")
                      drel_ap = bass.AP(drel_sb.tensor, drel_sb.offset + t_lo,
                                        [drel_sb.ap[0], [0, F], [1, nt]])
                      iota_ap = bass.AP(iota64.tensor, iota64.offset,
                                        [iota64.ap[0], [SGMAX, F], [1, nt]])
                      if "eq" not in skip:
                          nc.vector.tensor_tensor(out=p16[:], in0=drel_ap, in1=iota_ap,
                                                  op=mybir.AluOpType.is_equal)
                      else:
                          nc.vector.memset(p16[:, 0, 0:1], 0.0)
                      pd = epool.tile([128, F, nt], FP16, tag="pd")
                      if "pd" in skip:
                          nc.vector.memset(pd[:, 0, 0:1], 0.0)
                      for f in (range(f_lo, f_hi) if "pd" not in skip else []):
                          for (t0, fnt) in plan.frame_spans.get(f, []):
                              ft0 = t0 - t_lo
                              dm_ap = bass.AP(dmast.tensor, dmast.offset + f * F,
                                              [dmast.ap[0], [1, F], [0, fnt]])
                              nc.vector.tensor_tensor(
                                  out=pd[:, :, ft0:ft0 + fnt],
                                  in0=p16[:, :, ft0:ft0 + fnt],
                                  in1=dm_ap,
                                  op=mybir.AluOpType.mult)
                      dx = spool.tile([128, nt], FP16, tag="dx")
                      if "dx" in skip:
                          nc.vector.memset(dx[:], 0.0)
                      else:
                          pd_t = bass.AP(pd.tensor, pd.offset,
                                         [pd.ap[0], [1, nt], [nt, F]])
                          with nc.allow_low_precision(
                                  reason="one-hot masked sum: exactly one nonzero term"):
                              nc.vector.tensor_reduce(out=dx[:], in_=pd_t,
                                                      axis=mybir.AxisListType.X,
                                                      op=mybir.AluOpType.add)
                      eL = spool.tile([128, nt], FP16, tag="eL")
                      s_ap = bass.AP(g16.tensor, g16.offset + 128,
                                     [g16.ap[0], [RL, nt]])
                      nc.vector.tensor_tensor(out=eL[:], in0=s_ap, in1=dx[:],
                                              op=mybir.AluOpType.add)
                      ee = spool.tile([128, nt], FP16, tag="ee")
                      if "act" in skip:
                          nc.vector.memset(ee[:], 1.0)
                      else:
                          lr = spool.tile([128, nt], FP16, tag="lr")
                          nc.vector.scalar_tensor_tensor(
                              out=lr[:], in0=eL[:], scalar=NEG_SLOPE_ATT,
                              in1=eL[:], op0=mybir.AluOpType.mult,
                              op1=mybir.AluOpType.max)
                          nc.scalar.activation(out=ee[:], in_=lr[:],
                                               func=mybir.ActivationFunctionType.Exp)
                      pw = epool.tile([128, F, nt], FP16, tag="pw")
                      if "pw" not in skip:
                          ee_ap = bass.AP(ee.tensor, ee.offset,
                                          [ee.ap[0], [0, F], [1, nt]])
                          nc.vector.tensor_tensor(out=pw[:], in0=p16[:],
                                                  in1=ee_ap,
                                                  op=mybir.AluOpType.mult)
                      else:
                          nc.vector.memset(pw[:, 0, 0:1], 0.0)

                      for f in range(f_lo, f_hi):
                          spans = plan.frame_spans.get(f, [])
                          fnt = sum(n for (_, n) in spans)
                          if fnt == 0:
                              continue
                          acc = fps.tile([F, 130], FP32, space="PSUM", tag="acc",
                                         bufs=4)
                          if "scmm" in skip:
                              nc.vector.memset(acc[:, 0:1], 0.0)
                          else:
                              k = 0
                              for (t0, n) in spans:
                                  for j in range(n):
                                      ft = t0 - t_lo + j
                                      pw_j = bass.AP(pw.tensor, pw.offset + ft,
                                                     [pw.ap[0], [nt, F]])
                                      nc.tensor.matmul(
                                          out=acc[:], lhsT=pw_j,
                                          rhs=g16[:, ft, 0:130],
                                          start=(k == 0), stop=(k == fnt - 1))
                                      k += 1
                          nw = NL - f * F if f == NF - 1 and NF * F > NL else F
                          xslice = x64[0:nw, f, :]
                          if "norm" in skip:
                              nc.vector.tensor_copy(out=xslice, in_=acc[:nw, 0:128])
                              continue
                          zr = spool.tile([F, 1], FP32, tag="zr")
                          nc.vector.reciprocal(out=zr[:nw], in_=acc[:nw, 129:130])
                          if l == 1:
                              xr = spool.tile([F, D], FP32, tag="xr")
                              nc.vector.tensor_scalar(xr[:nw], acc[:nw, 0:128], zr[:nw],
                                                      None, mybir.AluOpType.mult)
                              nc.vector.scalar_tensor_tensor(
                                  out=xslice, in0=xr[:nw], scalar=NEG_SLOPE_ACT,
                                  in1=xr[:nw], op0=mybir.AluOpType.mult,
                                  op1=mybir.AluOpType.max)
                          else:
                              nc.vector.tensor_scalar(xslice, acc[:nw, 0:128], zr[:nw],
                                                      None, mybir.AluOpType.mult)

            # ---------- pooling ----------
            pool_ps = fps.tile([128, G], FP32, space="PSUM", tag="pool", bufs=1)
            for f in range(NF) if "pool" in phases else []:
                bt = bpool.tile([F, G], FP16, tag="bt")
                b_col = bass.AP(bslot_sb.tensor, bslot_sb.offset + f,
                                [bslot_sb.ap[0], [0, G]])
                nc.vector.tensor_tensor(out=bt[:], in0=b_col, in1=iota_g[:],
                                        op=mybir.AluOpType.is_equal)
                nc.tensor.matmul(out=pool_ps[:], lhsT=x64[:, f, :], rhs=bt[:],
                                 start=(f == 0), stop=(f == NF - 1))
            pooled_t = bpool.tile([128, G], FP32, tag="pooledt")  # [dim, graph]
            nc.vector.tensor_copy(out=pooled_t[:], in_=pool_ps[:])
            ar_in = dr.tile([128, G], FP32)
            ar_out = dr.tile([128, G], FP32,
                             addr_space="Local" if timing else "Shared")
            nc.sync.dma_start(out=ar_in[:], in_=pooled_t[:])
            if timing:
                nc.sync.dma_start(out=ar_out[:], in_=ar_in[:])
            else:
                nc.gpsimd.collective_compute(
                    "AllReduce", mybir.AluOpType.add,
                    replica_groups=[list(range(C))],
                    ins=[ar_in.opt()], outs=[ar_out.opt()],
                )
            red = bpool.tile([128, G], FP32, tag="red")
            nc.sync.dma_start(out=red[:], in_=ar_out[:])
            for gt in range(math.ceil(G / 128)):
                gcols = min(128, G - gt * 128)
                tps2 = bps.tile([128, 128], FP32, space="PSUM", tag="bps")
                nc.tensor.transpose(out=tps2[:gcols, :],
                                    in_=red[:, gt * 128:gt * 128 + gcols],
                                    identity=ident[:])
                og = bpool.tile([128, 128], FP32, tag="og")
                nc.vector.tensor_copy(out=og[:gcols, :], in_=tps2[:gcols, :])
                nc.sync.dma_start(out=out_ext[gt * 128:gt * 128 + gcols, :],
                                  in_=og[:gcols, :])
    nc.compile()
    return nc


_CACHE = {}


def _get_compiled(cfg: GATConfig, edge_index, batch):
    key = (cfg.n_nodes, cfg.n_edges, edge_index.tobytes()[:64], batch.tobytes()[:64])
    if key not in _CACHE:
        plan = build_edge_plan(cfg, edge_index)
        pools = build_pool_mats(cfg, batch)
        nc = build_bass(cfg, plan)
        _CACHE[key] = (nc, plan, pools)
    return _CACHE[key]


def build_in_maps(cfg: GATConfig, plan: EdgePlan, pools: list, inputs: dict):
    x = np.asarray(inputs["x"], np.float32)
    C, NL, NLP, D = cfg.n_cores, cfg.nloc, cfg.nloc_pad, cfg.dim
    in_maps = []
    for c in range(C):
        xs = np.zeros((D, NLP), np.float16)
        xs[:, :NL] = x[c * NL:(c + 1) * NL].T.astype(np.float16)
        m = {"xT": xs, "idx": plan.idx16[c], "dstrel": plan.dstrel[c],
             "bslot": pools[c], "gcnt": plan.gcnt[c]}
        for l in range(3):
            W = np.asarray(inputs[f"W{l}"], np.float32)
            a_s = np.asarray(inputs[f"a_src{l}"], np.float32)
            a_d = np.asarray(inputs[f"a_dst{l}"], np.float32)
            m[f"wt{l}"] = np.ascontiguousarray(W.T.astype(np.float16))
            m[f"ad{l}"] = np.ascontiguousarray(
                np.stack([W.T @ a_s, W.T @ a_d], axis=1).astype(np.float16))
        in_maps.append(m)
    return in_maps


def gat_forward(cfg: GATConfig, inputs: dict, trace: bool = False):
    edge_index = np.asarray(inputs["edge_index"])
    batch = np.asarray(inputs["batch"])
    nc, plan, pools = _get_compiled(cfg, edge_index, batch)
    in_maps = build_in_maps(cfg, plan, pools, inputs)
    r = run_bass_kernel_spmd(nc, in_maps, list(range(cfg.n_cores)), trace=trace)
    return (r.results[0]["out"], r) if trace else (r.results[0]["out"], None)


def kernel(**inputs) -> np.ndarray:
    cfg = GATConfig()
    out, _ = gat_forward(cfg, inputs)
    return out
